# revision 15
# baseline (speedup 1.0000x reference)
"""Trainium2 Bass kernel for nn_CrossIQ (3-modality cross attention), v2.

Reference computation (per batch b, modalities x0=rgb, x1=thermal, x2=depth):
    q_i = (wq_i @ x_i + bq_i) * s_i ; k_i = (wk_i @ x_i + bk_i) * s_i
    v_i = wv_i @ x_i + bv_i
    attend(q, ka, kb, v): a = softmax((q^T (ka+kb)) / 16, axis=m); out = v a^T
    w_rgb = attend(q0, k2, k1, v0); w_depth = attend(q2, k0, k1, v2)
    w_thermal = attend(q1, k0, k2, v1)
    fused = sum_i sigmoid(wg @ w_i + bg) * w_i ;  out = wo @ fused + bo

Key algebraic restructurings (all exact, verified vs reference):
  - The k biases contribute a per-n constant to every score row m, which
    cancels in the softmax over m -> k biases are DROPPED entirely.
  - The v bias passes through the softmax average (weights sum to 1):
    attn = raw_attn * (1/denom) + bv, applied in the epilogue.
  - ksum pairs: instead of 6 PSUM-accumulated pair convs (2 per attention),
    compute k0,k1,k2 once (3 convs) and form the 3 pairwise sums on DVE.

Softmax denominator: instead of one ones-matmul per m-chunk (18 extra PE
matmul streams per tile), sum pt chunks pairwise on DVE in bf16
(pairs -> quads -> eighths tree, ~160ns/op in 4x mode) and finish with 3
accumulating [1,NT] matmuls.

Everything is bf16 (weights, x, k-pairs, q, vt, pt) with fp32 PSUM; the
error budget (2e-2) comfortably covers it (~7e-3 measured). bf16 halves
DMA, enables FWL weight loads and 2x/4x DVE modes.

Sharding: 8 cores = (batch b in 0..3) x (query-half h in 0..1); the host
rotates each core's query half to the front so the SPMD program is
identical on all cores.
"""
import os
import sys
import types
import numpy as np
import ml_dtypes

# --- defensive shim: antenv.axon_hooks may be absent in this image; concourse
# imports it when trace=True under axon. Harmless no-op registration.
try:
    import antenv  # noqa: F401
    if "antenv.axon_hooks" not in sys.modules:
        _m = types.ModuleType("antenv.axon_hooks")
        _m._hook = None
        def _set(h):
            _m._hook = h
        def _get():
            return _m._hook
        _m.set_axon_ntff_profile_hook = _set
        _m.get_axon_ntff_profile_hook = _get
        sys.modules["antenv.axon_hooks"] = _m
        try:
            from trn_agent_boot.trn_boot import _ntff_profile_via_ctypes
            _h = _ntff_profile_via_ctypes("/opt/axon/libaxon_pjrt.so")
            if _h is not None:
                _m._hook = _h
        except Exception:
            pass
except Exception:
    pass

import concourse.bacc as bacc
import concourse.mybir as mybir
import concourse.tile as tile
from concourse.bass_utils import run_bass_kernel_spmd

B, C, H, W = 4, 256, 48, 48
N = H * W          # 2304 pixels (the m / key axis)
NH = N // 2        # 1152 query rows per core
NT = 384           # n-tile (<= 512 fp32 PSUM bank)
NNT = NH // NT     # 3 n-tiles
MC = N // 128      # 18 m-chunks
MT = 384           # m-tile for the k convs
NMT = N // MT      # 6
SCALE = 16.0       # sqrt(C)
LEAD = 6           # score chunks emitted ahead of the at-accumulation

DT_NAME = os.environ.get("KERNEL_DT", "bfloat16")

# attention spec: (alpha = q/v modality, beta/gamma = key modalities)
ATTN = [(0, 2, 1), (2, 0, 1), (1, 0, 2)]
# weight pack order (hot weights first: needed by the init phase)
W_ORDER = ["wk2", "wk1", "wq0", "wv0", "wk0", "wq2", "wv2", "wq1", "wv1",
           "wg", "wo"]
WIDX = {n: i for i, n in enumerate(W_ORDER)}
N_HOT = 5          # wk2, wk1, wq0, wv0, wk0 in the first weight DMA
# bias pack order ([128, idx, 2] f32)
B_ORDER = ["bq0", "bq1", "bq2", "bv0", "bv1", "bv2", "bgn", "bo"]
BIDX = {n: i for i, n in enumerate(B_ORDER)}

LAST_EXEC_NS = None
LAST_RESULTS = None

_CACHE = {}


def _dt():
    return getattr(mybir.dt, DT_NAME)


def _np_dt():
    return mybir.dt.np(_dt())


def build_bass():
    """Build the single-core program (identical on all 8 cores)."""
    DT = _dt()
    f32 = mybir.dt.float32
    AF = mybir.ActivationFunctionType
    nc = bacc.Bacc("TRN2", target_bir_lowering=False, debug=False)

    # ---- DRAM I/O ----
    xs_d = [nc.dram_tensor(f"x{i}", [128, 2, N], DT, kind="ExternalInput").ap()
            for i in range(3)]
    wpk_d = nc.dram_tensor("wpk", [128, 11, 2, C], DT, kind="ExternalInput").ap()
    bpk_d = nc.dram_tensor("bpk", [128, 8, 2], f32, kind="ExternalInput").ap()
    out_d = nc.dram_tensor("out", [128, 2, NH], f32, kind="ExternalOutput").ap()

    with tile.TileContext(nc) as tc:
        with (
            tc.tile_pool(name="consts", bufs=1) as consts,
            tc.tile_pool(name="q_p", bufs=2) as q_p,
            tc.tile_pool(name="vt_p", bufs=2) as vt_p,
            tc.tile_pool(name="pt_p", bufs=10) as pt_p,
            tc.tile_pool(name="pr_p", bufs=5) as pr_p,
            tc.tile_pool(name="qd_p", bufs=3) as qd_p,
            tc.tile_pool(name="e_p", bufs=2) as e_p,
            tc.tile_pool(name="attn_p", bufs=2) as attn_p,
            tc.tile_pool(name="small_p", bufs=2) as small_p,
            tc.tile_pool(name="eplg_p", bufs=2) as eplg_p,
            tc.tile_pool(name="out_p", bufs=2) as out_p,
            tc.tile_pool(name="pp_conv", bufs=2, space="PSUM") as pp_conv,
            tc.tile_pool(name="pp_sc", bufs=3, space="PSUM") as pp_sc,
            tc.tile_pool(name="pp_at", bufs=1, space="PSUM") as pp_at,
            tc.tile_pool(name="pp_dn", bufs=1, space="PSUM") as pp_dn,
        ):
            # ---- constant tiles ----
            wpk_sb = consts.tile([128, 11, 2, C], DT, tag="wpk")
            bpk_sb = consts.tile([128, 8, 2], f32, tag="bpk")
            x_sb = [consts.tile([128, 2, N], DT, tag=f"x{i}", name=f"x_sb{i}")
                    for i in range(3)]
            k_sb = {i: consts.tile([128, 2, N], DT, tag=f"k{i}", name=f"k_sb{i}")
                    for i in (1, 2)}
            pair_sb = [consts.tile([128, 2, N], DT, tag=f"pair{a}",
                                    name=f"pair_sb{a}") for a in range(3)]
            fused = consts.tile([128, 2, NH], f32, tag="fused")
            ones_col = consts.tile([128, 4], DT, tag="ones_col")
            ones_row = consts.tile([1, 128], DT, tag="ones_row")

            # ---- input DMAs: few large transfers, spread over the 3 DMA
            # queues (sync/SP, scalar/ACT, gpsimd); wk2 rides alone first so
            # the very first conv matmul can start asap ----
            XS = 768  # x split point: [0:768] first (covers conv m-tiles 0,1)
            nc.sync.dma_start(out=wpk_sb[:, 0:1], in_=wpk_d[:, 0:1])
            nc.scalar.dma_start(out=x_sb[2][:, :, 0:XS], in_=xs_d[2][:, :, 0:XS])
            nc.gpsimd.dma_start(out=bpk_sb[:], in_=bpk_d[:])
            nc.sync.dma_start(out=wpk_sb[:, 1:N_HOT], in_=wpk_d[:, 1:N_HOT])
            nc.gpsimd.dma_start(out=x_sb[1][:, :, 0:XS], in_=xs_d[1][:, :, 0:XS])
            nc.scalar.dma_start(out=x_sb[2][:, :, XS:N], in_=xs_d[2][:, :, XS:N])
            nc.gpsimd.dma_start(out=x_sb[0][:, :, 0:XS], in_=xs_d[0][:, :, 0:XS])
            nc.scalar.dma_start(out=x_sb[1][:, :, XS:N], in_=xs_d[1][:, :, XS:N])
            nc.sync.dma_start(out=x_sb[0][:, :, XS:N], in_=xs_d[0][:, :, XS:N])
            nc.gpsimd.dma_start(out=wpk_sb[:, N_HOT:11], in_=wpk_d[:, N_HOT:11])

            nc.vector.memset(ones_col[:], 1.0)
            nc.vector.memset(ones_row[:], 1.0)
            nc.vector.memset(fused[:], 0.0)

            def wsl(name, ci, cols=slice(0, C)):
                return wpk_sb[:, WIDX[name], ci, cols]

            def bsl(name, co):
                return bpk_sb[:, BIDX[name], co:co + 1]

            # ---------------- conv-step machinery ----------------
            # Each step is a closure emitting one small instruction group.
            # Steps are drained a couple per attention m-chunk so conv work
            # for the next attention interleaves with the current one.
            steps = []

            def drain(k=1):
                for _ in range(k):
                    if steps:
                        steps.pop(0)()

            def k2k1_step(mt):
                # k2 + k1 convs for one (mt) over both co, then the pair0
                # pieces; incremental so attention 0 can start after mt=0,1
                def go(mt=mt):
                    mcols = slice(mt * MT, (mt + 1) * MT)
                    for co in range(2):
                        pks = {}
                        for ki in (2, 1):
                            pk = pp_conv.tile([128, MT], f32, tag="convp",
                                              name=f"pk{ki}_{co}_{mt}")
                            for ci in range(2):
                                nc.tensor.matmul(
                                    pk[:],
                                    wsl(f"wk{ki}", ci, slice(co * 128, (co + 1) * 128)),
                                    x_sb[ki][:, ci, mcols],
                                    start=(ci == 0), stop=(ci == 1),
                                )
                            nc.scalar.activation(
                                k_sb[ki][:, co, mcols], pk[:], AF.Copy)
                            pks[ki] = pk
                        nc.vector.tensor_add(
                            pair_sb[0][:, co, mcols],
                            k_sb[2][:, co, mcols], k_sb[1][:, co, mcols])
                return go

            def k0_step(mt):
                # k0 conv for one (mt), both co; forms pair1 (k0+k1) and
                # pair2 (k0+k2) pieces straight from PSUM (k0 never lands
                # in SBUF on its own)
                def go(mt=mt):
                    mcols = slice(mt * MT, (mt + 1) * MT)
                    for co in range(2):
                        pk = pp_conv.tile([128, MT], f32, tag="convp",
                                          name=f"pk0_{co}_{mt}")
                        for ci in range(2):
                            nc.tensor.matmul(
                                pk[:],
                                wsl("wk0", ci, slice(co * 128, (co + 1) * 128)),
                                x_sb[0][:, ci, mcols],
                                start=(ci == 0), stop=(ci == 1),
                            )
                        nc.vector.tensor_add(
                            pair_sb[1][:, co, mcols], pk[:],
                            k_sb[1][:, co, mcols])
                        nc.vector.tensor_add(
                            pair_sb[2][:, co, mcols], pk[:],
                            k_sb[2][:, co, mcols])
                return go

            def q_step(a, co, nt):
                al = ATTN[a][0]

                def go(al=al, co=co, nt=nt):
                    ncols = slice(nt * NT, (nt + 1) * NT)
                    pq = pp_conv.tile([128, NT], f32, tag="convp",
                                      name=f"pq{al}_{co}_{nt}")
                    for ci in range(2):
                        nc.tensor.matmul(
                            pq[:],
                            wsl(f"wq{al}", ci, slice(co * 128, (co + 1) * 128)),
                            x_sb[al][:, ci, ncols],
                            start=(ci == 0), stop=(ci == 1),
                        )
                    nc.scalar.activation(
                        q_tiles[a][:, co, ncols], pq[:], AF.Identity,
                        bias=bsl(f"bq{al}", co))
                return go

            def v_step(a, mp):
                # two m-chunks per step: the [128, 2, C] f32 PSUM tile is
                # exactly one bank, and one ACT copy moves both chunks
                al = ATTN[a][0]

                def go(al=al, a=a, mp=mp):
                    pv = pp_conv.tile([128, 2, C], f32, tag="convp",
                                      name=f"pv{al}_{mp}")
                    for mi in range(2):
                        m = 2 * mp + mi
                        for ci in range(2):
                            nc.tensor.matmul(
                                pv[:, mi, :],
                                x_sb[al][:, ci, slice(m * 128, (m + 1) * 128)],
                                wsl(f"wv{al}", ci),
                                start=(ci == 0), stop=(ci == 1),
                            )
                    nc.scalar.activation(
                        vt_tiles[a][:, 2 * mp:2 * mp + 2, :], pv[:], AF.Copy)
                return go

            q_tiles = {}
            vt_tiles = {}

            def alloc_proj(a):
                q_tiles[a] = q_p.tile([128, 2, NH], DT, tag="q", name=f"q{a}")
                vt_tiles[a] = vt_p.tile([128, MC, C], DT, tag="vt", name=f"vt{a}")

            # ---- init phase: minimum needed to start attention 0 ----
            alloc_proj(0)
            k2k1_step(0)()
            k2k1_step(1)()
            q_step(0, 0, 0)()
            q_step(0, 1, 0)()
            # remaining conv work for attention 0, drained during tile (0,0):
            # step at queue position i is drained at the TOP of chunk i, so
            # k2k1_step(mt) must sit at position <= 3*mt (its pair piece is
            # read by the chunk-3*mt score MM) and v_step(0, mp) at position
            # <= 2*mp+LEAD (read by the at-matmul of chunk 2*mp).
            init_steps = {3: k2k1_step(2), 7: k2k1_step(3),
                          11: k2k1_step(4), 15: k2k1_step(5)}
            vq = [v_step(0, mp) for mp in range(MC // 2)] + \
                 [q_step(0, co, 1) for co in range(2)]
            for pos in range(24):
                st = init_steps.get(pos) or (vq.pop(0) if vq else None)
                steps.append(st or (lambda: None))

            def queue_phase(a):
                """Queue projection steps for attention a (a >= 1)."""
                alloc_proj(a)
                if a == 1:
                    for co in range(2):
                        steps.append(q_step(0, co, 2))
                    for mt in range(NMT):
                        steps.append(k0_step(mt))
                for co, nt in ((0, 0), (1, 0)):
                    steps.append(q_step(a, co, nt))
                for mp in range(MC // 2):
                    steps.append(v_step(a, mp))
                for co, nt in ((0, 1), (1, 1), (0, 2), (1, 2)):
                    steps.append(q_step(a, co, nt))

            queue_phase(1)

            deferred = []

            def flush_deferred():
                while deferred:
                    deferred.pop(0)()

            # ---- output conv (per-nt once the last attention's epilogue
            # for that column range has flushed) ----
            def out_conv(nt):
                ncols = slice(nt * NT, (nt + 1) * NT)
                fdt = out_p.tile([128, 2, NT], DT, tag="fdt", name=f"fdt{nt}")
                nc.vector.tensor_copy(fdt[:, 0, :], fused[:, 0, ncols])
                nc.vector.tensor_copy(fdt[:, 1, :], fused[:, 1, ncols])
                for co in range(2):
                    po = pp_conv.tile([128, NT], f32, tag="convp",
                                      name=f"po{nt}_{co}")
                    for ci in range(2):
                        nc.tensor.matmul(
                            po[:],
                            wsl("wo", ci, slice(co * 128, (co + 1) * 128)),
                            fdt[:, ci, :],
                            start=(ci == 0), stop=(ci == 1),
                        )
                    osb = out_p.tile([128, NT], f32, tag="osb",
                                     name=f"osb{nt}_{co}")
                    nc.vector.tensor_scalar_add(osb[:], po[:], bsl("bo", co))
                    nc.sync.dma_start(out=out_d[:, co, ncols], in_=osb[:])

            # ---------------- attention ----------------
            for a, (al, be, ga) in enumerate(ATTN):
                for nt in range(NNT):
                    scopeB = nc.named_scope(f"phB_{a}_{nt}")
                    scopeB.__enter__()
                    flush_deferred()
                    if a == 2 and nt >= 1:
                        out_conv(nt - 1)
                    if a == 1 and nt == 0:
                        queue_phase(2)
                    ncols = slice(nt * NT, (nt + 1) * NT)
                    at0 = pp_at.tile([128, NT], f32, tag="at0",
                                     name=f"at0_{a}_{nt}")
                    at1 = pp_at.tile([128, NT], f32, tag="at1",
                                     name=f"at1_{a}_{nt}")
                    dn = pp_dn.tile([1, NT], f32, tag="dn", name=f"dn{a}_{nt}")
                    pr_mt = pr_p.tile([128, 9, NT], DT, tag="pr",
                                      name=f"pr{a}_{nt}")
                    qd_mt = qd_p.tile([128, 4, NT], DT, tag="qd",
                                      name=f"qd{a}_{nt}")
                    e_mt = e_p.tile([128, 2, NT], DT, tag="e",
                                    name=f"e{a}_{nt}")
                    pts = {}
                    # drain pacing: tile (0,0) must drain exactly one queued
                    # step per chunk (ordering constraints); later tiles
                    # spread the remaining conv work more thinly
                    dev = 1 if (a, nt) == (0, 0) else (2 if a == 0 else 3)
                    for m in range(MC + LEAD):
                        # drain BEFORE emitting this chunk's reads: a step's
                        # writes must precede any same-tile consumer emission
                        if m % dev == 0:
                            drain(1)
                        if m < MC:
                            sc_ps = pp_sc.tile([128, NT], f32, tag="sc",
                                               name=f"sc{a}_{nt}_{m}")
                            for ci in range(2):
                                nc.tensor.matmul(
                                    sc_ps[:],
                                    pair_sb[a][:, ci, m * 128:(m + 1) * 128],
                                    q_tiles[a][:, ci, ncols],
                                    start=(ci == 0), stop=(ci == 1),
                                )
                            pt = pt_p.tile([128, NT], DT, tag="pt",
                                           name=f"pt{a}_{nt}_{m}")
                            nc.scalar.activation(pt[:], sc_ps[:], AF.Exp)
                            pts[m] = pt
                            # denominator reduction tree (DVE, 2x bf16):
                            # chunk pairs individually (fine-grained deps),
                            # then one strided op per upper level
                            if m % 2 == 1:
                                nc.vector.tensor_add(
                                    pr_mt[:, m // 2, :], pts[m - 1][:], pts[m][:])
                            if m == 15:
                                nc.vector.tensor_add(
                                    qd_mt[:], pr_mt[:, 0:8:2, :],
                                    pr_mt[:, 1:8:2, :])
                                nc.vector.tensor_add(
                                    e_mt[:], qd_mt[:, 0:4:2, :],
                                    qd_mt[:, 1:4:2, :])
                        if m >= LEAD:
                            j = m - LEAD
                            nc.tensor.matmul(at0[:], vt_tiles[a][:, j, 0:128],
                                             pts[j][:],
                                             start=(j == 0), stop=(j == MC - 1))
                            nc.tensor.matmul(at1[:], vt_tiles[a][:, j, 128:256],
                                             pts[j][:],
                                             start=(j == 0), stop=(j == MC - 1))
                    # final denominator accumulation: e0 + e1 + pr8
                    for i, t in enumerate((e_mt[:, 0, :], e_mt[:, 1, :],
                                           pr_mt[:, 8, :])):
                        nc.tensor.matmul(dn[:], ones_col[:, i:i + 1], t,
                                         start=(i == 0), stop=(i == 2))
                    r_sb = small_p.tile([1, NT], f32, tag="r",
                                        name=f"r{a}_{nt}")
                    nc.vector.reciprocal_approx_fast(r_sb[:], dn[:])

                    # epilogue deferred until the next tile's scores are
                    # emitted, so the PE chews on those while this
                    # DVE/ACT chain drains
                    def epilogue(a=a, al=al, nt=nt, ncols=ncols, at0=at0,
                                 at1=at1, r_sb=r_sb):
                        rdt = small_p.tile([1, NT], DT, tag="rdt",
                                           name=f"rdt{a}_{nt}")
                        nc.vector.tensor_copy(rdt[:], r_sb[:])
                        rb = pp_conv.tile([128, NT], f32, tag="convp",
                                          name=f"rb{a}_{nt}")
                        nc.tensor.matmul(rb[:], ones_row[:], rdt[:])
                        rb_sb = small_p.tile([128, NT], DT, tag="rb_sb",
                                             name=f"rbs{a}_{nt}")
                        nc.vector.tensor_copy(rb_sb[:], rb[:])
                        attn = attn_p.tile([128, 2, NT], DT, tag="attn",
                                           name=f"attn{a}_{nt}")
                        for co, at in ((0, at0), (1, at1)):
                            nc.vector.tensor_mul(attn[:, co, :], at[:], rb_sb[:])
                            nc.vector.tensor_scalar_add(
                                attn[:, co, :], attn[:, co, :],
                                bsl(f"bv{al}", co))
                        # gate: fused += sigmoid(wg attn + bg) * attn
                        for co in range(2):
                            pg = pp_conv.tile([128, NT], f32, tag="convp",
                                              name=f"pg{a}_{nt}_{co}")
                            for ci in range(2):
                                nc.tensor.matmul(
                                    pg[:],
                                    wsl("wg", ci, slice(co * 128, (co + 1) * 128)),
                                    attn[:, ci, :],
                                    start=(ci == 0), stop=(ci == 1),
                                )
                            # sigmoid(x) = 1/(1+exp(-x)); Exp keeps the ACT
                            # engine on a single function table
                            en = eplg_p.tile([128, NT], f32, tag="en",
                                             name=f"en{a}_{nt}_{co}")
                            nc.scalar.activation(
                                en[:], pg[:], AF.Exp,
                                bias=bsl("bgn", co), scale=-1.0)
                            nc.vector.tensor_scalar_add(en[:], en[:], 1.0)
                            nc.vector.reciprocal_approx_fast(en[:], en[:])
                            gt = eplg_p.tile([128, NT], f32, tag="gt",
                                             name=f"gt{a}_{nt}_{co}")
                            nc.vector.tensor_mul(gt[:], en[:], attn[:, co, :])
                            nc.vector.tensor_add(
                                fused[:, co, ncols], fused[:, co, ncols], gt[:])

                    scopeB.__exit__(None, None, None)
                    deferred.append(epilogue)

            flush_deferred()
            out_conv(2)

    nc.compile()
    return nc


def _pack_chw(arr):
    """[256, X] f32 -> [128, 2, X] in DT order (c_lo, c_hi, X)."""
    return np.ascontiguousarray(
        arr.reshape(2, 128, -1).transpose(1, 0, 2)).astype(_np_dt())


def _pack_bias(b):
    """[256] -> [128, 2] f32 (c_lo, c_hi)."""
    return np.ascontiguousarray(b.reshape(2, 128).T).astype(np.float32)


def _pack_w(w, scale=1.0):
    """[c_out, c_in] -> lhsT layout [128, 2, 256] = (c_in_lo, c_in_hi, c_out)."""
    wt = (w.astype(np.float64) * scale).astype(np.float32).T  # [c_in, c_out]
    return np.ascontiguousarray(
        wt.reshape(2, 128, C).transpose(1, 0, 2)).astype(_np_dt())


def kernel(**inputs):
    global LAST_EXEC_NS, LAST_RESULTS
    inp = {k: np.asarray(v) for k, v in inputs.items()}
    s = inp["s"].astype(np.float32)

    if "nc" not in _CACHE:
        _CACHE["nc"] = build_bass()
    nc = _CACHE["nc"]

    # ---- host-side packing ----
    wmap = {}
    for i in range(3):
        wmap[f"wq{i}"] = _pack_w(inp[f"wq{i}"], s[i] / SCALE)
        wmap[f"wk{i}"] = _pack_w(inp[f"wk{i}"], s[i])
        wmap[f"wv{i}"] = _pack_w(inp[f"wv{i}"])
    wmap["wg"] = _pack_w(inp["wg"])
    wmap["wo"] = _pack_w(inp["wo"])
    wpk = np.stack([wmap[n] for n in W_ORDER], axis=1)  # [128, 11, 2, 256]

    bmap = {}
    for i in range(3):
        bmap[f"bq{i}"] = _pack_bias(inp[f"bq{i}"].astype(np.float32)
                                    * (s[i] / SCALE))
        bmap[f"bv{i}"] = _pack_bias(inp[f"bv{i}"])
    bmap["bgn"] = _pack_bias(-inp["bg"].astype(np.float32))
    bmap["bo"] = _pack_bias(inp["bo"])
    bpk = np.stack([bmap[n] for n in B_ORDER], axis=1)  # [128, 8, 2]

    shared = {"wpk": np.ascontiguousarray(wpk),
              "bpk": np.ascontiguousarray(bpk)}

    in_maps = []
    for core in range(8):
        b, h = core // 2, core % 2
        m = dict(shared)
        for i in range(3):
            xp = _pack_chw(inp[f"x{i}"][b].reshape(C, N).astype(np.float32))
            if h == 1:  # rotate so this core's query half comes first
                xp = np.ascontiguousarray(
                    np.concatenate([xp[:, :, NH:], xp[:, :, :NH]], axis=2))
            m[f"x{i}"] = xp
        in_maps.append(m)

    trace = bool(os.environ.get("BASS_TRACE"))
    res = run_bass_kernel_spmd(nc, in_maps, core_ids=list(range(8)), trace=trace)
    LAST_EXEC_NS = res.exec_time_ns
    LAST_RESULTS = res

    out = np.empty((B, C, N), np.float32)
    for core in range(8):
        b, h = core // 2, core % 2
        o = res.results[core]["out"]  # [128, 2, NH] f32
        out[b, :, h * NH:(h + 1) * NH] = o.transpose(1, 0, 2).reshape(C, NH)
    return out.reshape(B, C, H, W)


# revision 18
# speedup vs baseline: 1.0143x; 1.0143x over previous
"""Trainium2 Bass kernel for nn_CrossIQ (3-modality cross attention), v2.

Reference computation (per batch b, modalities x0=rgb, x1=thermal, x2=depth):
    q_i = (wq_i @ x_i + bq_i) * s_i ; k_i = (wk_i @ x_i + bk_i) * s_i
    v_i = wv_i @ x_i + bv_i
    attend(q, ka, kb, v): a = softmax((q^T (ka+kb)) / 16, axis=m); out = v a^T
    w_rgb = attend(q0, k2, k1, v0); w_depth = attend(q2, k0, k1, v2)
    w_thermal = attend(q1, k0, k2, v1)
    fused = sum_i sigmoid(wg @ w_i + bg) * w_i ;  out = wo @ fused + bo

Key algebraic restructurings (all exact, verified vs reference):
  - The k biases contribute a per-n constant to every score row m, which
    cancels in the softmax over m -> k biases are DROPPED entirely.
  - The v bias passes through the softmax average (weights sum to 1):
    attn = raw_attn * (1/denom) + bv, applied in the epilogue.
  - ksum pairs: instead of 6 PSUM-accumulated pair convs (2 per attention),
    compute k0,k1,k2 once (3 convs) and form the 3 pairwise sums on DVE.

Softmax denominator: instead of one ones-matmul per m-chunk (18 extra PE
matmul streams per tile), sum pt chunks pairwise on DVE in bf16
(pairs -> quads -> eighths tree, ~160ns/op in 4x mode) and finish with 3
accumulating [1,NT] matmuls.

Everything is bf16 (weights, x, k-pairs, q, vt, pt) with fp32 PSUM; the
error budget (2e-2) comfortably covers it (~7e-3 measured). bf16 halves
DMA, enables FWL weight loads and 2x/4x DVE modes.

Sharding: 8 cores = (batch b in 0..3) x (query-half h in 0..1); the host
rotates each core's query half to the front so the SPMD program is
identical on all cores.
"""
import os
import sys
import types
import numpy as np
import ml_dtypes

# --- defensive shim: antenv.axon_hooks may be absent in this image; concourse
# imports it when trace=True under axon. Harmless no-op registration.
try:
    import antenv  # noqa: F401
    if "antenv.axon_hooks" not in sys.modules:
        _m = types.ModuleType("antenv.axon_hooks")
        _m._hook = None
        def _set(h):
            _m._hook = h
        def _get():
            return _m._hook
        _m.set_axon_ntff_profile_hook = _set
        _m.get_axon_ntff_profile_hook = _get
        sys.modules["antenv.axon_hooks"] = _m
        try:
            from trn_agent_boot.trn_boot import _ntff_profile_via_ctypes
            _h = _ntff_profile_via_ctypes("/opt/axon/libaxon_pjrt.so")
            if _h is not None:
                _m._hook = _h
        except Exception:
            pass
except Exception:
    pass

import concourse.bacc as bacc
import concourse.mybir as mybir
import concourse.tile as tile
from concourse.bass_utils import run_bass_kernel_spmd

B, C, H, W = 4, 256, 48, 48
N = H * W          # 2304 pixels (the m / key axis)
NH = N // 2        # 1152 query rows per core
NT = 384           # n-tile (<= 512 fp32 PSUM bank)
NNT = NH // NT     # 3 n-tiles
MC = N // 128      # 18 m-chunks
MT = 384           # m-tile for the k convs
NMT = N // MT      # 6
SCALE = 16.0       # sqrt(C)
LEAD = 8           # score chunks emitted ahead of the at-accumulation

DT_NAME = os.environ.get("KERNEL_DT", "bfloat16")

# attention spec: (alpha = q/v modality, beta/gamma = key modalities)
ATTN = [(0, 2, 1), (2, 0, 1), (1, 0, 2)]
# weight pack order (hot weights first: needed by the init phase)
W_ORDER = ["wk2", "wk1", "wq0", "wv0", "wk0", "wq2", "wv2", "wq1", "wv1",
           "wg", "wo"]
WIDX = {n: i for i, n in enumerate(W_ORDER)}
N_HOT = 5          # wk2, wk1, wq0, wv0, wk0 in the first weight DMA
# bias pack order ([128, idx, 2] f32)
B_ORDER = ["bq0", "bq1", "bq2", "bv0", "bv1", "bv2", "bgn", "bo"]
BIDX = {n: i for i, n in enumerate(B_ORDER)}

LAST_EXEC_NS = None
LAST_RESULTS = None

_CACHE = {}


def _dt():
    return getattr(mybir.dt, DT_NAME)


def _np_dt():
    return mybir.dt.np(_dt())


def build_bass():
    """Build the single-core program (identical on all 8 cores)."""
    DT = _dt()
    f32 = mybir.dt.float32
    AF = mybir.ActivationFunctionType
    nc = bacc.Bacc("TRN2", target_bir_lowering=False, debug=False)

    # ---- DRAM I/O ----
    xs_d = [nc.dram_tensor(f"x{i}", [128, 2, N], DT, kind="ExternalInput").ap()
            for i in range(3)]
    wpk_d = nc.dram_tensor("wpk", [128, 11, 2, C], DT, kind="ExternalInput").ap()
    bpk_d = nc.dram_tensor("bpk", [128, 8, 2], f32, kind="ExternalInput").ap()
    out_d = nc.dram_tensor("out", [128, 2, NH], f32, kind="ExternalOutput").ap()

    with tile.TileContext(nc) as tc:
        with (
            tc.tile_pool(name="consts", bufs=1) as consts,
            tc.tile_pool(name="q_p", bufs=2) as q_p,
            tc.tile_pool(name="vt_p", bufs=2) as vt_p,
            tc.tile_pool(name="pt_p", bufs=12) as pt_p,
            tc.tile_pool(name="pr_p", bufs=5) as pr_p,
            tc.tile_pool(name="qd_p", bufs=3) as qd_p,
            tc.tile_pool(name="e_p", bufs=2) as e_p,
            tc.tile_pool(name="attn_p", bufs=2) as attn_p,
            tc.tile_pool(name="small_p", bufs=2) as small_p,
            tc.tile_pool(name="eplg_p", bufs=2) as eplg_p,
            tc.tile_pool(name="out_p", bufs=2) as out_p,
            tc.tile_pool(name="pp_conv", bufs=2, space="PSUM") as pp_conv,
            tc.tile_pool(name="pp_sc", bufs=3, space="PSUM") as pp_sc,
            tc.tile_pool(name="pp_at", bufs=1, space="PSUM") as pp_at,
            tc.tile_pool(name="pp_dn", bufs=1, space="PSUM") as pp_dn,
        ):
            # ---- constant tiles ----
            wpk_sb = consts.tile([128, 11, 2, C], DT, tag="wpk")
            bpk_sb = consts.tile([128, 8, 2], f32, tag="bpk")
            x_sb = [consts.tile([128, 2, N], DT, tag=f"x{i}", name=f"x_sb{i}")
                    for i in range(3)]
            k_sb = {i: consts.tile([128, 2, N], DT, tag=f"k{i}", name=f"k_sb{i}")
                    for i in (1, 2)}
            pair_sb = [consts.tile([128, 2, N], DT, tag=f"pair{a}",
                                    name=f"pair_sb{a}") for a in range(3)]
            fused = consts.tile([128, 2, NH], f32, tag="fused")
            ones_col = consts.tile([128, 4], DT, tag="ones_col")
            ones_row = consts.tile([1, 128], DT, tag="ones_row")

            # ---- input DMAs: per-queue DMA bandwidth is only ~120GB/s, so
            # split into just-in-time pieces round-robined over the 3 DMA
            # queues (sync/SP, scalar/ACT, gpsimd), ordered by first use ----
            def xdma(eng, i, lo, hi):
                eng.dma_start(out=x_sb[i][:, :, lo:hi], in_=xs_d[i][:, :, lo:hi])

            nc.sync.dma_start(out=wpk_sb[:, 0:2], in_=wpk_d[:, 0:2])  # wk2,wk1
            nc.scalar.dma_start(out=x_sb[2][:, :, 0:384], in_=xs_d[2][:, :, 0:384])
            nc.gpsimd.dma_start(out=bpk_sb[:], in_=bpk_d[:])
            xdma(nc.gpsimd, 1, 0, 384)
            nc.sync.dma_start(out=wpk_sb[:, 2:4], in_=wpk_d[:, 2:4])  # wq0,wv0
            xdma(nc.scalar, 2, 384, 768)
            xdma(nc.gpsimd, 1, 384, 768)
            xdma(nc.sync, 0, 0, 384)
            xdma(nc.scalar, 2, 768, 1536)
            xdma(nc.gpsimd, 1, 768, 1536)
            xdma(nc.sync, 0, 384, 1152)
            xdma(nc.scalar, 0, 1152, N)
            xdma(nc.gpsimd, 1, 1536, N)
            nc.sync.dma_start(out=wpk_sb[:, 4:5], in_=wpk_d[:, 4:5])  # wk0
            xdma(nc.scalar, 2, 1536, N)
            nc.gpsimd.dma_start(out=wpk_sb[:, N_HOT:11], in_=wpk_d[:, N_HOT:11])

            nc.vector.memset(ones_col[:], 1.0)
            nc.vector.memset(ones_row[:], 1.0)
            nc.vector.memset(fused[:], 0.0)

            def wsl(name, ci, cols=slice(0, C)):
                return wpk_sb[:, WIDX[name], ci, cols]

            def bsl(name, co):
                return bpk_sb[:, BIDX[name], co:co + 1]

            # ---------------- conv-step machinery ----------------
            # Each step is a closure emitting one small instruction group.
            # Steps are drained a couple per attention m-chunk so conv work
            # for the next attention interleaves with the current one.
            steps = []

            def drain(k=1):
                for _ in range(k):
                    if steps:
                        steps.pop(0)()

            def k2k1_step(mt):
                # k2 + k1 convs for one (mt) over both co, then the pair0
                # pieces; incremental so attention 0 can start after mt=0,1
                def go(mt=mt):
                    mcols = slice(mt * MT, (mt + 1) * MT)
                    for co in range(2):
                        pks = {}
                        for ki in (2, 1):
                            pk = pp_conv.tile([128, MT], f32, tag="convp",
                                              name=f"pk{ki}_{co}_{mt}")
                            for ci in range(2):
                                nc.tensor.matmul(
                                    pk[:],
                                    wsl(f"wk{ki}", ci, slice(co * 128, (co + 1) * 128)),
                                    x_sb[ki][:, ci, mcols],
                                    start=(ci == 0), stop=(ci == 1),
                                )
                            nc.scalar.activation(
                                k_sb[ki][:, co, mcols], pk[:], AF.Copy)
                            pks[ki] = pk
                        nc.vector.tensor_add(
                            pair_sb[0][:, co, mcols],
                            k_sb[2][:, co, mcols], k_sb[1][:, co, mcols])
                return go

            def k0_step(mt):
                # k0 conv for one (mt), both co; forms pair1 (k0+k1) and
                # pair2 (k0+k2) pieces straight from PSUM (k0 never lands
                # in SBUF on its own)
                def go(mt=mt):
                    mcols = slice(mt * MT, (mt + 1) * MT)
                    for co in range(2):
                        pk = pp_conv.tile([128, MT], f32, tag="convp",
                                          name=f"pk0_{co}_{mt}")
                        for ci in range(2):
                            nc.tensor.matmul(
                                pk[:],
                                wsl("wk0", ci, slice(co * 128, (co + 1) * 128)),
                                x_sb[0][:, ci, mcols],
                                start=(ci == 0), stop=(ci == 1),
                            )
                        nc.vector.tensor_add(
                            pair_sb[1][:, co, mcols], pk[:],
                            k_sb[1][:, co, mcols])
                        nc.vector.tensor_add(
                            pair_sb[2][:, co, mcols], pk[:],
                            k_sb[2][:, co, mcols])
                return go

            def q_step(a, co, nt):
                al = ATTN[a][0]

                def go(al=al, co=co, nt=nt):
                    ncols = slice(nt * NT, (nt + 1) * NT)
                    pq = pp_conv.tile([128, NT], f32, tag="convp",
                                      name=f"pq{al}_{co}_{nt}")
                    for ci in range(2):
                        nc.tensor.matmul(
                            pq[:],
                            wsl(f"wq{al}", ci, slice(co * 128, (co + 1) * 128)),
                            x_sb[al][:, ci, ncols],
                            start=(ci == 0), stop=(ci == 1),
                        )
                    nc.scalar.activation(
                        q_tiles[a][:, co, ncols], pq[:], AF.Identity,
                        bias=bsl(f"bq{al}", co))
                return go

            def v_step(a, mp):
                # two m-chunks per step: the [128, 2, C] f32 PSUM tile is
                # exactly one bank, and one ACT copy moves both chunks
                al = ATTN[a][0]

                def go(al=al, a=a, mp=mp):
                    pv = pp_conv.tile([128, 2, C], f32, tag="convp",
                                      name=f"pv{al}_{mp}")
                    for mi in range(2):
                        m = 2 * mp + mi
                        for ci in range(2):
                            nc.tensor.matmul(
                                pv[:, mi, :],
                                x_sb[al][:, ci, slice(m * 128, (m + 1) * 128)],
                                wsl(f"wv{al}", ci),
                                start=(ci == 0), stop=(ci == 1),
                            )
                    nc.scalar.activation(
                        vt_tiles[a][:, 2 * mp:2 * mp + 2, :], pv[:], AF.Copy)
                return go

            q_tiles = {}
            vt_tiles = {}

            def alloc_proj(a):
                q_tiles[a] = q_p.tile([128, 2, NH], DT, tag="q", name=f"q{a}")
                vt_tiles[a] = vt_p.tile([128, MC, C], DT, tag="vt", name=f"vt{a}")

            # ---- init phase: minimum needed to start attention 0 ----
            alloc_proj(0)
            k2k1_step(0)()
            k2k1_step(1)()
            q_step(0, 0, 0)()
            q_step(0, 1, 0)()
            # remaining conv work for attention 0, drained during tile (0,0):
            # step at queue position i is drained at the TOP of chunk i, so
            # k2k1_step(mt) must sit at position <= 3*mt (its pair piece is
            # read by the chunk-3*mt score MM) and v_step(0, mp) at position
            # <= 2*mp+LEAD (read by the at-matmul of chunk 2*mp).
            init_steps = {3: k2k1_step(2), 7: k2k1_step(3),
                          11: k2k1_step(4), 15: k2k1_step(5)}
            vq = [v_step(0, mp) for mp in range(MC // 2)] + \
                 [q_step(0, co, 1) for co in range(2)]
            for pos in range(24):
                st = init_steps.get(pos) or (vq.pop(0) if vq else None)
                steps.append(st or (lambda: None))

            def queue_phase(a):
                """Queue projection steps for attention a (a >= 1)."""
                alloc_proj(a)
                if a == 1:
                    for co in range(2):
                        steps.append(q_step(0, co, 2))
                    for mt in range(NMT):
                        steps.append(k0_step(mt))
                for co, nt in ((0, 0), (1, 0)):
                    steps.append(q_step(a, co, nt))
                for mp in range(MC // 2):
                    steps.append(v_step(a, mp))
                for co, nt in ((0, 1), (1, 1), (0, 2), (1, 2)):
                    steps.append(q_step(a, co, nt))

            queue_phase(1)

            deferred = []

            def flush_deferred():
                while deferred:
                    deferred.pop(0)()

            # ---- output conv (per-nt once the last attention's epilogue
            # for that column range has flushed) ----
            def out_conv(nt):
                ncols = slice(nt * NT, (nt + 1) * NT)
                fdt = out_p.tile([128, 2, NT], DT, tag="fdt", name=f"fdt{nt}")
                nc.vector.tensor_copy(fdt[:, 0, :], fused[:, 0, ncols])
                nc.vector.tensor_copy(fdt[:, 1, :], fused[:, 1, ncols])
                for co in range(2):
                    po = pp_conv.tile([128, NT], f32, tag="convp",
                                      name=f"po{nt}_{co}")
                    for ci in range(2):
                        nc.tensor.matmul(
                            po[:],
                            wsl("wo", ci, slice(co * 128, (co + 1) * 128)),
                            fdt[:, ci, :],
                            start=(ci == 0), stop=(ci == 1),
                        )
                    osb = out_p.tile([128, NT], f32, tag="osb",
                                     name=f"osb{nt}_{co}")
                    nc.vector.tensor_scalar_add(osb[:], po[:], bsl("bo", co))
                    nc.sync.dma_start(out=out_d[:, co, ncols], in_=osb[:])

            # ---------------- attention ----------------
            for a, (al, be, ga) in enumerate(ATTN):
                for nt in range(NNT):
                    scopeB = nc.named_scope(f"phB_{a}_{nt}")
                    scopeB.__enter__()
                    flush_deferred()
                    if a == 2 and nt >= 1:
                        out_conv(nt - 1)
                    if a == 1 and nt == 0:
                        queue_phase(2)
                    ncols = slice(nt * NT, (nt + 1) * NT)
                    at0 = pp_at.tile([128, NT], f32, tag="at0",
                                     name=f"at0_{a}_{nt}")
                    at1 = pp_at.tile([128, NT], f32, tag="at1",
                                     name=f"at1_{a}_{nt}")
                    dn = pp_dn.tile([1, NT], f32, tag="dn", name=f"dn{a}_{nt}")
                    pr_mt = pr_p.tile([128, 9, NT], DT, tag="pr",
                                      name=f"pr{a}_{nt}")
                    qd_mt = qd_p.tile([128, 4, NT], DT, tag="qd",
                                      name=f"qd{a}_{nt}")
                    e_mt = e_p.tile([128, 2, NT], DT, tag="e",
                                    name=f"e{a}_{nt}")
                    pts = {}
                    # drain pacing: tile (0,0) must drain exactly one queued
                    # step per chunk (ordering constraints); later tiles
                    # spread the remaining conv work more thinly
                    dev = 1 if (a, nt) == (0, 0) else (2 if a == 0 else 3)
                    for m in range(MC + LEAD):
                        # drain BEFORE emitting this chunk's reads: a step's
                        # writes must precede any same-tile consumer emission
                        if m % dev == 0:
                            drain(1)
                        if m < MC:
                            sc_ps = pp_sc.tile([128, NT], f32, tag="sc",
                                               name=f"sc{a}_{nt}_{m}")
                            for ci in range(2):
                                nc.tensor.matmul(
                                    sc_ps[:],
                                    pair_sb[a][:, ci, m * 128:(m + 1) * 128],
                                    q_tiles[a][:, ci, ncols],
                                    start=(ci == 0), stop=(ci == 1),
                                )
                            pt = pt_p.tile([128, NT], DT, tag="pt",
                                           name=f"pt{a}_{nt}_{m}")
                            nc.scalar.activation(pt[:], sc_ps[:], AF.Exp)
                            pts[m] = pt
                            # denominator reduction tree (DVE, 2x bf16):
                            # chunk pairs individually (fine-grained deps),
                            # then one strided op per upper level
                            if m % 2 == 1:
                                nc.vector.tensor_add(
                                    pr_mt[:, m // 2, :], pts[m - 1][:], pts[m][:])
                            if m == 15:
                                nc.vector.tensor_add(
                                    qd_mt[:], pr_mt[:, 0:8:2, :],
                                    pr_mt[:, 1:8:2, :])
                                nc.vector.tensor_add(
                                    e_mt[:], qd_mt[:, 0:4:2, :],
                                    qd_mt[:, 1:4:2, :])
                        if m >= LEAD:
                            j = m - LEAD
                            nc.tensor.matmul(at0[:], vt_tiles[a][:, j, 0:128],
                                             pts[j][:],
                                             start=(j == 0), stop=(j == MC - 1))
                            nc.tensor.matmul(at1[:], vt_tiles[a][:, j, 128:256],
                                             pts[j][:],
                                             start=(j == 0), stop=(j == MC - 1))
                    # final denominator accumulation: e0 + e1 + pr8
                    for i, t in enumerate((e_mt[:, 0, :], e_mt[:, 1, :],
                                           pr_mt[:, 8, :])):
                        nc.tensor.matmul(dn[:], ones_col[:, i:i + 1], t,
                                         start=(i == 0), stop=(i == 2))
                    r_sb = small_p.tile([1, NT], f32, tag="r",
                                        name=f"r{a}_{nt}")
                    nc.vector.reciprocal_approx_fast(r_sb[:], dn[:])

                    # epilogue deferred until the next tile's scores are
                    # emitted, so the PE chews on those while this
                    # DVE/ACT chain drains
                    def epilogue(a=a, al=al, nt=nt, ncols=ncols, at0=at0,
                                 at1=at1, r_sb=r_sb):
                        rdt = small_p.tile([1, NT], DT, tag="rdt",
                                           name=f"rdt{a}_{nt}")
                        nc.vector.tensor_copy(rdt[:], r_sb[:])
                        rb = pp_conv.tile([128, NT], f32, tag="convp",
                                          name=f"rb{a}_{nt}")
                        nc.tensor.matmul(rb[:], ones_row[:], rdt[:])
                        rb_sb = small_p.tile([128, NT], DT, tag="rb_sb",
                                             name=f"rbs{a}_{nt}")
                        nc.vector.tensor_copy(rb_sb[:], rb[:])
                        attn = attn_p.tile([128, 2, NT], DT, tag="attn",
                                           name=f"attn{a}_{nt}")
                        for co, at in ((0, at0), (1, at1)):
                            nc.vector.tensor_mul(attn[:, co, :], at[:], rb_sb[:])
                            nc.vector.tensor_scalar_add(
                                attn[:, co, :], attn[:, co, :],
                                bsl(f"bv{al}", co))
                        # gate: fused += sigmoid(wg attn + bg) * attn
                        for co in range(2):
                            pg = pp_conv.tile([128, NT], f32, tag="convp",
                                              name=f"pg{a}_{nt}_{co}")
                            for ci in range(2):
                                nc.tensor.matmul(
                                    pg[:],
                                    wsl("wg", ci, slice(co * 128, (co + 1) * 128)),
                                    attn[:, ci, :],
                                    start=(ci == 0), stop=(ci == 1),
                                )
                            # sigmoid(x) = 1/(1+exp(-x)); Exp keeps the ACT
                            # engine on a single function table
                            en = eplg_p.tile([128, NT], f32, tag="en",
                                             name=f"en{a}_{nt}_{co}")
                            nc.scalar.activation(
                                en[:], pg[:], AF.Exp,
                                bias=bsl("bgn", co), scale=-1.0)
                            nc.vector.tensor_scalar_add(en[:], en[:], 1.0)
                            nc.vector.reciprocal_approx_fast(en[:], en[:])
                            gt = eplg_p.tile([128, NT], f32, tag="gt",
                                             name=f"gt{a}_{nt}_{co}")
                            nc.vector.tensor_mul(gt[:], en[:], attn[:, co, :])
                            nc.vector.tensor_add(
                                fused[:, co, ncols], fused[:, co, ncols], gt[:])

                    scopeB.__exit__(None, None, None)
                    deferred.append(epilogue)

            flush_deferred()
            out_conv(2)

    nc.compile()
    return nc


def _pack_chw(arr):
    """[256, X] f32 -> [128, 2, X] in DT order (c_lo, c_hi, X)."""
    return np.ascontiguousarray(
        arr.reshape(2, 128, -1).transpose(1, 0, 2)).astype(_np_dt())


def _pack_bias(b):
    """[256] -> [128, 2] f32 (c_lo, c_hi)."""
    return np.ascontiguousarray(b.reshape(2, 128).T).astype(np.float32)


def _pack_w(w, scale=1.0):
    """[c_out, c_in] -> lhsT layout [128, 2, 256] = (c_in_lo, c_in_hi, c_out)."""
    wt = (w.astype(np.float64) * scale).astype(np.float32).T  # [c_in, c_out]
    return np.ascontiguousarray(
        wt.reshape(2, 128, C).transpose(1, 0, 2)).astype(_np_dt())


def kernel(**inputs):
    global LAST_EXEC_NS, LAST_RESULTS
    inp = {k: np.asarray(v) for k, v in inputs.items()}
    s = inp["s"].astype(np.float32)

    if "nc" not in _CACHE:
        _CACHE["nc"] = build_bass()
    nc = _CACHE["nc"]

    # ---- host-side packing ----
    wmap = {}
    for i in range(3):
        wmap[f"wq{i}"] = _pack_w(inp[f"wq{i}"], s[i] / SCALE)
        wmap[f"wk{i}"] = _pack_w(inp[f"wk{i}"], s[i])
        wmap[f"wv{i}"] = _pack_w(inp[f"wv{i}"])
    wmap["wg"] = _pack_w(inp["wg"])
    wmap["wo"] = _pack_w(inp["wo"])
    wpk = np.stack([wmap[n] for n in W_ORDER], axis=1)  # [128, 11, 2, 256]

    bmap = {}
    for i in range(3):
        bmap[f"bq{i}"] = _pack_bias(inp[f"bq{i}"].astype(np.float32)
                                    * (s[i] / SCALE))
        bmap[f"bv{i}"] = _pack_bias(inp[f"bv{i}"])
    bmap["bgn"] = _pack_bias(-inp["bg"].astype(np.float32))
    bmap["bo"] = _pack_bias(inp["bo"])
    bpk = np.stack([bmap[n] for n in B_ORDER], axis=1)  # [128, 8, 2]

    shared = {"wpk": np.ascontiguousarray(wpk),
              "bpk": np.ascontiguousarray(bpk)}

    in_maps = []
    for core in range(8):
        b, h = core // 2, core % 2
        m = dict(shared)
        for i in range(3):
            xp = _pack_chw(inp[f"x{i}"][b].reshape(C, N).astype(np.float32))
            if h == 1:  # rotate so this core's query half comes first
                xp = np.ascontiguousarray(
                    np.concatenate([xp[:, :, NH:], xp[:, :, :NH]], axis=2))
            m[f"x{i}"] = xp
        in_maps.append(m)

    trace = bool(os.environ.get("BASS_TRACE"))
    res = run_bass_kernel_spmd(nc, in_maps, core_ids=list(range(8)), trace=trace)
    LAST_EXEC_NS = res.exec_time_ns
    LAST_RESULTS = res

    out = np.empty((B, C, N), np.float32)
    for core in range(8):
        b, h = core // 2, core % 2
        o = res.results[core]["out"]  # [128, 2, NH] f32
        out[b, :, h * NH:(h + 1) * NH] = o.transpose(1, 0, 2).reshape(C, NH)
    return out.reshape(B, C, H, W)
